# revision 25
# baseline (speedup 1.0000x reference)
"""CapsuleLayer dynamic-routing kernel for 8 trn2 NeuronCores.

Sharding: route nodes (N=2048) split across 8 cores (256 each); x and W
sharded by n, weights never replicated.  Per routing iteration the only
cross-core exchange is an AllReduce of the 64x32x32 fp32 partial sum s.

Per-core dataflow (bf16 operands, fp32 PSUM):
  iter0:  s0 = (1/J) sum_{n,i} x*W           K=(n,i) matmuls, W_C layout
  iter t: Wv[b,n,j,i] = sum_o W*v            K=(jq,o) matmuls, W_B layout
          xWv = Wv * x                       DVE TT fused with PSUM evac
          a[b,n,j] = sum_i xWv               ones-blockdiag matmul
          c = softmax_j(blogits)             ACT exp + DVE trees
          cx = c*x                           DVE TT with broadcast APs
          s_t = sum_{n,i} cx*W               K=(n,i) matmuls, diag extract
          AllReduce(s) ; v = squash(s)
"""

import os
import re
import sys

for _p in ("/opt/trn_rl_repo", "/root/.axon_site/_ro/trn_rl_repo"):
    if os.path.isdir(_p) and _p not in sys.path:
        sys.path.insert(0, _p)

import numpy as np
import ml_dtypes

import concourse.bass as bass
import concourse.bacc as bacc
import concourse.mybir as mybir
from concourse import tile
from concourse.vector_clock import ScopedClock
import bass_rust

BF16 = mybir.dt.bfloat16
FP32 = mybir.dt.float32
AF = mybir.ActivationFunctionType
ALU = mybir.AluOpType

N_CORES = 8
ROUTING_ITERS = 3


def _patch_tile_drain():
    """This walrus build allows only one sync wait per instruction; Tile's
    kernel-tail drain packs the whole vector clock onto one Drain.  Split it
    into one drain per outstanding proc."""

    def _drain_and_barrier(self, tick_clock, wait_clock):
        gc = tick_clock.global_clock
        ticks = eval(re.sub(r"VectorClock", "", repr(gc)))
        n = len(ticks)
        for i, v in enumerate(ticks):
            if v > 0:
                single = [0] * n
                single[i] = v
                d = self.nc.sync.drain()
                wait_clock.add_sem_waits(
                    d.ins, ScopedClock({None: bass_rust.VectorClock(single)})
                )
        self.nc.all_engine_barrier()
        popped = self.nc._tile_sem_poison_stack.pop()
        assert popped is self._sem_poison
        self.nc.clear_and_free_semaphores(list(self.sems.allocated().values()))
        self.nc.all_engine_barrier()

    tile.TileContext._drain_and_barrier = _drain_and_barrier


_patch_tile_drain()


def _dims(B, NLOC, I, J, O):
    JQ = 4
    JG = J // JQ
    PK = JQ * O            # stage-1 contraction rows
    NN = min(128 // I, NLOC)
    PC = NN * I            # (n,i) chunk partition rows
    NCH = NLOC // NN
    PN = min(128, NLOC)    # n rows per half
    NH = NLOC // PN
    CHH = NCH // NH
    assert JG * JQ == J and NN * NCH == NLOC and PN * NH == NLOC
    return JQ, JG, PK, NN, PC, NCH, PN, NH, CHH


def build_kernel(B=64, NLOC=256, I=16, J=32, O=32, n_cores=N_CORES):
    """Emit the per-core SPMD program.  Returns the Bass module."""
    JQ, JG, PK, NN, PC, NCH, PN, NH, CHH = _dims(B, NLOC, I, J, O)
    JO = J * O
    IJO = I * J * O

    nc = bacc.Bacc("TRN2", target_bir_lowering=False, debug=False,
                   num_devices=n_cores)

    # ---- dram parameters (host-prepped layouts) ----
    wb_d = nc.declare_dram_parameter("w_b", [JG * PK, NLOC * I], BF16,
                                     isOutput=False)
    wc_d = nc.declare_dram_parameter("w_c", [NLOC, IJO], BF16, isOutput=False)
    x2_d = nc.declare_dram_parameter("x2", [NLOC, I * B], BF16, isOutput=False)
    xin_d = nc.declare_dram_parameter("x_i_n", [NCH * PC, B], BF16,
                                      isOutput=False)
    ones_d = nc.declare_dram_parameter("ones_bd", [PC, NN], BF16,
                                       isOutput=False)
    out_d = nc.declare_dram_parameter("v_out", [B, JO], FP32, isOutput=True)

    # collective bounce buffers (internal dram)
    ar_space = "Shared" if n_cores > 4 else "Local"
    # all bounces are [B, JO]; t>=1 in bf16 (partials transposed on-chip
    # with the xbar before the AllReduce)
    ar_in = [nc.dram_tensor(f"ar_in{t}", [B, JO],
                            FP32 if t == 0 else BF16)
             for t in range(ROUTING_ITERS)]
    ar_out = [nc.dram_tensor(f"ar_out{t}", [B, JO],
                             FP32 if t == 0 else BF16, addr_space=ar_space)
              for t in range(ROUTING_ITERS)]
    blog_d = nc.dram_tensor("blog_spill", [NLOC, J * B], FP32)

    rg = [list(range(n_cores))]

    with tile.TileContext(nc) as tc:
        with (
            tc.tile_pool(name="wpool", bufs=1) as wpool,
            tc.tile_pool(name="small", bufs=1) as small,
            tc.tile_pool(name="work", bufs=2) as work,
            tc.tile_pool(name="cxp", bufs=2) as cxp,
            tc.tile_pool(name="xwvp", bufs=2) as xwvp,
            tc.tile_pool(name="achk", bufs=3) as achk,
            tc.tile_pool(name="ps", bufs=2, space="PSUM") as ps,
            tc.tile_pool(name="ps_wv", bufs=2, space="PSUM") as ps_wv,
            tc.tile_pool(name="ps_a", bufs=1, space="PSUM") as ps_a,
        ):
            # ---- resident tensors ----
            wc_t = []
            for h in range(NH):
                t = wpool.tile([PN, IJO], BF16, tag=f"wc{h}")
                nc.sync.dma_start(out=t[:, :], in_=wc_d[h * PN:(h + 1) * PN, :])
                wc_t.append(t)
            x2_t = []
            for h in range(NH):
                t = wpool.tile([PN, I * B], BF16, tag=f"x2_{h}")
                nc.sync.dma_start(out=t[:, :], in_=x2_d[h * PN:(h + 1) * PN, :])
                x2_t.append(t)
            xin_t = []
            for ch in range(NCH):
                t = wpool.tile([PC, B], BF16, tag=f"xin{ch}")
                nc.sync.dma_start(out=t[:, :], in_=xin_d[ch * PC:(ch + 1) * PC, :])
                xin_t.append(t)
            ones_t = wpool.tile([PC, NN], BF16, tag="ones")
            nc.sync.dma_start(out=ones_t[:, :], in_=ones_d[:, :])
            wb_t = []
            for jg in range(JG):
                t = wpool.tile([PK, NLOC * I], BF16, tag=f"wb{jg}")
                nc.sync.dma_start(out=t[:, :], in_=wb_d[jg * PK:(jg + 1) * PK, :])
                wb_t.append(t)

            # v-blockdiag rhs tiles for stage-1 (zero background, diag
            # rewritten per iteration)
            vblk = []
            for jg in range(JG):
                t = wpool.tile([PK, JQ * B], BF16, tag=f"vblk{jg}")
                nc.vector.memset(t[:, :], 0.0)
                vblk.append(t)

            v_bf = small.tile([B, JO], BF16, tag="v_bf")
            s_sb = small.tile([B, JO], FP32, tag="s_sb")

            # ---------- squash: s_sb -> v (bf16) ----------
            def squash(final=False):
                sq = work.tile([B, JO], FP32, tag="sq", bufs=1)
                nc.vector.tensor_tensor(out=sq[:, :], in0=s_sb[:, :],
                                        in1=s_sb[:, :], op=ALU.mult)
                norm = work.tile([B, J], FP32, tag="norm")
                nc.vector.tensor_reduce(
                    out=norm[:, :].unsqueeze(2),
                    in_=sq[:, :].rearrange("p (j o) -> p j o", o=O),
                    axis=mybir.AxisListType.X, op=ALU.add)
                np1 = work.tile([B, J], FP32, tag="np1")
                nc.vector.tensor_scalar_add(out=np1[:, :], in0=norm[:, :],
                                            scalar1=1.0)
                r1 = work.tile([B, J], FP32, tag="r1")
                nc.vector.reciprocal(out=r1[:, :], in_=np1[:, :])
                ne = work.tile([B, J], FP32, tag="ne")
                nc.vector.tensor_scalar_add(out=ne[:, :], in0=norm[:, :],
                                            scalar1=1e-8)
                sr = work.tile([B, J], FP32, tag="sr")
                nc.scalar.activation(sr[:, :], ne[:, :], AF.Sqrt)
                r2 = work.tile([B, J], FP32, tag="r2")
                nc.vector.reciprocal(out=r2[:, :], in_=sr[:, :])
                sc = work.tile([B, J], FP32, tag="sc")
                nc.vector.tensor_tensor(out=sc[:, :], in0=norm[:, :],
                                        in1=r1[:, :], op=ALU.mult)
                nc.vector.tensor_tensor(out=sc[:, :], in0=sc[:, :],
                                        in1=r2[:, :], op=ALU.mult)
                sc_b = sc[:, :].unsqueeze(2).broadcast_to((B, J, O))
                s3 = s_sb[:, :].rearrange("p (j o) -> p j o", o=O)
                if final:
                    vf = work.tile([B, JO], FP32, tag="sq", bufs=1, name="vf")
                    nc.vector.tensor_tensor(
                        out=vf[:, :].rearrange("p (j o) -> p j o", o=O),
                        in0=s3, in1=sc_b, op=ALU.mult)
                    nc.sync.dma_start(out=out_d[:, :], in_=vf[:, :])
                else:
                    nc.vector.tensor_tensor(
                        out=v_bf[:, :].rearrange("p (j o) -> p j o", o=O),
                        in0=s3, in1=sc_b, op=ALU.mult)

            # ---------- iter 0: s0 ----------
            nsplit = 512 if JO > 512 else JO
            s0_ps = [ps.tile([B, nsplit], FP32, tag="sps", name=f"s0ps{k}")
                     for k in range(JO // nsplit)]
            first = True
            for h in range(NH):
                for i in range(I):
                    lhsT = x2_t[h][:, i * B:(i + 1) * B]
                    for k in range(JO // nsplit):
                        nc.tensor.matmul(
                            s0_ps[k][:, :], lhsT,
                            wc_t[h][:, i * JO + k * nsplit: i * JO + (k + 1) * nsplit],
                            start=first, stop=(h == NH - 1 and i == I - 1))
                    first = False
            for k in range(JO // nsplit):
                nc.vector.tensor_scalar_mul(
                    out=s_sb[:, k * nsplit:(k + 1) * nsplit],
                    in0=s0_ps[k][:, :], scalar1=1.0 / J)
            nc.sync.dma_start(out=ar_in[0][:, :], in_=s_sb[:, :])
            nc.gpsimd.collective_compute(
                "AllReduce", ALU.add, replica_groups=rg,
                ins=[ar_in[0][:, :]], outs=[ar_out[0][:, :]])
            nc.sync.dma_start(out=s_sb[:, :], in_=ar_out[0][:, :])
            squash()

            # ---------- routing iterations ----------
            for t in range(1, ROUTING_ITERS):
                # v -> transposed slices -> block-diag rhs tiles
                for jg in range(JG):
                    vT = work.tile([PK, B], BF16, tag="vT", bufs=2,
                                   name=f"vT{t}_{jg}")
                    nc.sync.dma_start_transpose(
                        vT[:, :], v_bf[:, jg * PK:(jg + 1) * PK])
                    for jq in range(JQ):
                        nc.vector.tensor_copy(
                            out=vblk[jg][jq * O:(jq + 1) * O,
                                         jq * B:(jq + 1) * B],
                            in_=vT[jq * O:(jq + 1) * O, :])

                # agreement logits per n-half.  Chunks processed in pairs:
                # stage-2 matmuls for the two chunks run concurrently in
                # separate PE column-groups (tile_position), and one wide
                # ScalarE copy evacuates both.
                QW = min(4, JG)          # j-groups per wide psum tile
                evi = 0
                c_bf = []
                for h in range(NH):
                    asb = work.tile([PN, J * B], FP32, tag="asb", bufs=2,
                                    name=f"asb{t}_{h}")
                    for chh0 in range(0, CHH, 2):
                        npack = min(2, CHH - chh0)
                        xwvs = []
                        for pi in range(npack):
                            chh = chh0 + pi
                            ch = h * CHH + chh
                            for qw in range(JG // QW):
                                wv_ps = ps_wv.tile(
                                    [PC, QW * JQ * B], FP32, tag="wvps",
                                    bufs=2, name=f"wvps{t}_{ch}_{qw}")
                                for jj in range(QW):
                                    jg = qw * QW + jj
                                    nc.tensor.matmul(
                                        wv_ps[:, jj * JQ * B:(jj + 1) * JQ * B],
                                        wb_t[jg][:, ch * PC:(ch + 1) * PC],
                                        vblk[jg][:, :], start=True, stop=True)
                                # evacuate + multiply by x.  Rotate the
                                # evac across engines: most go straight
                                # through DVE (TT from PSUM); some bounce
                                # via a ScalarE cast-copy so GpSimd/DVE can
                                # do the multiply from SBUF at 2x.
                                xwv = xwvp.tile([PC, QW * JQ * B], BF16,
                                                tag="xwv", bufs=3,
                                                name=f"xwv{t}_{ch}_{qw}")
                                xb = xin_t[ch][:, :].unsqueeze(1).unsqueeze(1) \
                                    .broadcast_to((PC, QW, JQ, B))
                                xwv4 = xwv[:, :].rearrange(
                                    "p (g q b) -> p g q b", q=JQ, b=B)
                                mode = evi % 8
                                evi += 1
                                if mode >= 5:
                                    wvs = xwvp.tile([PC, QW * JQ * B], BF16,
                                                    tag="wvs", bufs=2,
                                                    name=f"wvs{t}_{ch}_{qw}")
                                    nc.scalar.activation(wvs[:, :], wv_ps[:, :],
                                                         AF.Copy)
                                    meng = nc.gpsimd if mode <= 6 else nc.vector
                                    meng.tensor_tensor(
                                        out=xwv4,
                                        in0=wvs[:, :].rearrange(
                                            "p (g q b) -> p g q b", q=JQ, b=B),
                                        in1=xb, op=ALU.mult)
                                else:
                                    wv4 = wv_ps[:, :].rearrange(
                                        "p (g q b) -> p g q b", q=JQ, b=B)
                                    nc.vector.tensor_tensor(
                                        out=xwv4, in0=wv4, in1=xb, op=ALU.mult)
                                xwvs.append(xwv)
                        # stage-2: ones-blockdiag reduce over i; the pack's
                        # chunks land in different psum partition groups
                        nw = JG * JQ * B
                        ksplit = min(1024, nw)
                        ap_rows = 64 if npack == 2 else NN
                        for k in range(nw // ksplit):
                            a_ps = ps_a.tile([ap_rows, ksplit], FP32,
                                             tag="aps", bufs=1,
                                             name=f"aps{t}_{h}_{chh0}_{k}")
                            msplit = min(512, ksplit)
                            for pi in range(npack):
                                for m in range(ksplit // msplit):
                                    cof = k * ksplit + m * msplit
                                    xw = xwvs[pi * (JG // QW) + cof // (QW * JQ * B)]
                                    koff = cof % (QW * JQ * B)
                                    nc.tensor.matmul(
                                        a_ps[32 * pi:32 * pi + NN,
                                             m * msplit:(m + 1) * msplit],
                                        ones_t[:, :],
                                        xw[:, koff:koff + msplit],
                                        start=True, stop=True,
                                        tile_position=(0, 32 * pi) if pi else None)
                            ach = achk.tile([ap_rows, ksplit], FP32,
                                            tag="ach", bufs=2,
                                            name=f"ach{t}_{h}_{chh0}_{k}")
                            nc.scalar.activation(ach[:, :], a_ps[:, :],
                                                 AF.Copy)
                            for pi in range(npack):
                                chh = chh0 + pi
                                nc.sync.dma_start(
                                    out=asb[chh * NN:(chh + 1) * NN,
                                            k * ksplit:(k + 1) * ksplit],
                                    in_=ach[32 * pi:32 * pi + NN, :])
                    if t == 1:
                        nc.sync.dma_start(
                            out=blog_d[h * PN:(h + 1) * PN, :], in_=asb[:, :])
                    else:
                        bl = work.tile([PN, J * B], FP32, tag="asb", bufs=2,
                                       name=f"bl{t}_{h}")
                        nc.sync.dma_start(
                            out=bl[:, :], in_=blog_d[h * PN:(h + 1) * PN, :])
                        nc.vector.tensor_tensor(out=asb[:, :], in0=asb[:, :],
                                                in1=bl[:, :], op=ALU.add)
                    # softmax over j (no max subtraction; logits are O(1));
                    # exp in place, bf16 scratch for the denominator tree
                    nc.scalar.activation(asb[:, :], asb[:, :], AF.Exp)
                    cb = work.tile([PN, J * B], BF16, tag="cb", bufs=2,
                                   name=f"cb{t}_{h}")
                    nc.vector.tensor_copy(out=cb[:, :], in_=asb[:, :])
                    w = J
                    while w > 1:
                        hw = w // 2
                        s3 = cb[:, :].rearrange("p (j b) -> p j b", b=B)
                        nc.vector.tensor_tensor(
                            out=s3[:, 0:hw, :], in0=s3[:, 0:hw, :],
                            in1=s3[:, hw:w, :], op=ALU.add)
                        w = hw
                    re_ = work.tile([PN, B], FP32, tag="re", bufs=2,
                                    name=f"re{t}_{h}")
                    nc.vector.reciprocal(
                        out=re_[:, :],
                        in_=cb[:, :].rearrange("p (j b) -> p j b", b=B)[:, 0:1, :].squeeze(1))
                    nc.vector.tensor_tensor(
                        out=cb[:, :].rearrange("p (j b) -> p j b", b=B),
                        in0=asb[:, :].rearrange("p (j b) -> p j b", b=B),
                        in1=re_[:, :].unsqueeze(1).broadcast_to((PN, J, B)),
                        op=ALU.mult)
                    c_bf.append(cb)

                # s_t = sum_{n,i} cx * W   (cx built in i-halves, build work
                # split between VectorE and GpSimd)
                IH = I // 2 if I % 2 == 0 else I
                cxi = 0
                for jg in range(JG):
                    s_ps = ps.tile([PK, JQ * B], FP32, tag="sps", bufs=2,
                                   name=f"sps{t}_{jg}")
                    for h in range(NH):
                        for ih in range(I // IH):
                            cx = cxp.tile([PN, IH * JQ * B], BF16, tag="cx",
                                          bufs=2, name=f"cx{t}_{jg}_{h}_{ih}")
                            i0 = ih * IH
                            c_ap = c_bf[h][:, :].rearrange(
                                "p (j b) -> p j b", b=B)[:, jg * JQ:(jg + 1) * JQ, :]
                            c_ap = c_ap.unsqueeze(1).broadcast_to(
                                (PN, IH, JQ, B))
                            x_ap = x2_t[h][:, i0 * B:(i0 + IH) * B].rearrange(
                                "p (i b) -> p i b", b=B).unsqueeze(2).broadcast_to(
                                (PN, IH, JQ, B))
                            eng = nc.gpsimd if cxi % 3 == 2 else nc.vector
                            cxi += 1
                            eng.tensor_tensor(
                                out=cx[:, :].rearrange(
                                    "p (i q b) -> p i q b", q=JQ, b=B),
                                in0=c_ap, in1=x_ap, op=ALU.mult)
                            for ii in range(IH):
                                i = i0 + ii
                                nc.tensor.matmul(
                                    s_ps[:, :],
                                    wc_t[h][:, i * JO + jg * PK: i * JO + jg * PK + PK],
                                    cx[:, ii * JQ * B:(ii + 1) * JQ * B],
                                    start=(h == 0 and i == 0),
                                    stop=(h == NH - 1 and i == I - 1))
                    # extract diagonal blocks into a jg-pair tile, then
                    # xbar-transpose to [B, (j,o)] for the AllReduce bounce
                    if jg % 2 == 0:
                        sdp = work.tile([PK, 2 * B], BF16, tag="sd", bufs=2,
                                        name=f"sd{t}_{jg}")
                    half = (jg % 2) * B
                    for jq in range(JQ):
                        nc.vector.tensor_copy(
                            out=sdp[jq * O:(jq + 1) * O, half:half + B],
                            in_=s_ps[jq * O:(jq + 1) * O,
                                     jq * B:(jq + 1) * B])
                    if jg % 2 == 1 or jg == JG - 1:
                        sdT = work.tile([2 * B, PK], BF16, tag="sdT", bufs=2,
                                        name=f"sdT{t}_{jg}")
                        nc.sync.dma_start_transpose(sdT[:, :], sdp[:, :])
                        for g2 in range(jg % 2 + 1):
                            jgw = jg - (jg % 2) + g2
                            nc.sync.dma_start(
                                out=ar_in[t][:, jgw * PK:(jgw + 1) * PK],
                                in_=sdT[g2 * B:(g2 + 1) * B, :])
                nc.gpsimd.collective_compute(
                    "AllReduce", ALU.add, replica_groups=rg,
                    ins=[ar_in[t][:, :]], outs=[ar_out[t][:, :]])
                sT = work.tile([B, JO], BF16, tag="sT", bufs=1,
                               name=f"sT{t}")
                nc.sync.dma_start(out=sT[:, :], in_=ar_out[t][:, :])
                nc.vector.tensor_copy(out=s_sb[:, :], in_=sT[:, :])
                squash(final=(t == ROUTING_ITERS - 1))

    nc.compile()
    return nc


def prep_inputs(x, weights, n_cores=N_CORES):
    """Shard + lay out the inputs for each core."""
    B, N, I = x.shape
    _, J, O, _ = weights.shape
    NLOC = N // n_cores
    JQ, JG, PK, NN, PC, NCH, PN, NH, CHH = _dims(B, NLOC, I, J, O)
    bf = ml_dtypes.bfloat16
    ones = np.zeros((PC, NN), dtype=bf)
    for nn in range(NN):
        ones[nn * I:(nn + 1) * I, nn] = 1.0
    x = np.asarray(x, dtype=np.float32)
    weights = np.asarray(weights, dtype=np.float32)
    in_maps = []
    for r in range(n_cores):
        n0 = r * NLOC
        Wr = weights[n0:n0 + NLOC]              # [NLOC, J, O, I]
        xr = x[:, n0:n0 + NLOC, :]              # [B, NLOC, I]
        w_b = Wr.reshape(NLOC, JG, JQ, O, I).transpose(1, 2, 3, 0, 4) \
            .reshape(JG * JQ * O, NLOC * I).astype(bf)
        w_c = Wr.transpose(0, 3, 1, 2).reshape(NLOC, I * J * O).astype(bf)
        x_nib = xr.transpose(1, 2, 0)           # [NLOC, I, B]
        x2 = x_nib.reshape(NLOC, I * B).astype(bf)
        x_i_n = x_nib.reshape(NCH * PC, B).astype(bf)
        in_maps.append({
            "w_b": np.ascontiguousarray(w_b),
            "w_c": np.ascontiguousarray(w_c),
            "x2": np.ascontiguousarray(x2),
            "x_i_n": np.ascontiguousarray(x_i_n),
            "ones_bd": ones,
        })
    return in_maps


_CACHE = {}


def kernel(x, weights):
    from concourse.bass_utils import run_bass_kernel_spmd
    x = np.asarray(x)
    weights = np.asarray(weights)
    B, N, I = x.shape
    _, J, O, _ = weights.shape
    NLOC = N // N_CORES
    key = (B, N, I, J, O)
    if key not in _CACHE:
        _CACHE[key] = build_kernel(B=B, NLOC=NLOC, I=I, J=J, O=O)
    nc = _CACHE[key]
    in_maps = prep_inputs(x, weights)
    res = run_bass_kernel_spmd(nc, in_maps, list(range(N_CORES)))
    out = np.asarray(res.results[0]["v_out"], dtype=np.float32)
    return out.reshape(B, J, O)


# revision 28
# speedup vs baseline: 1.0364x; 1.0364x over previous
"""CapsuleLayer dynamic-routing kernel for 8 trn2 NeuronCores.

Sharding: route nodes (N=2048) split across 8 cores (256 each); x and W
sharded by n, weights never replicated.  Per routing iteration the only
cross-core exchange is an AllReduce of the 64x32x32 fp32 partial sum s.

Per-core dataflow (bf16 operands, fp32 PSUM):
  iter0:  s0 = (1/J) sum_{n,i} x*W           K=(n,i) matmuls, W_C layout
  iter t: Wv[b,n,j,i] = sum_o W*v            K=(jq,o) matmuls, W_B layout
          xWv = Wv * x                       DVE TT fused with PSUM evac
          a[b,n,j] = sum_i xWv               ones-blockdiag matmul
          c = softmax_j(blogits)             ACT exp + DVE trees
          cx = c*x                           DVE TT with broadcast APs
          s_t = sum_{n,i} cx*W               K=(n,i) matmuls, diag extract
          AllReduce(s) ; v = squash(s)
"""

import os
import re
import sys

for _p in ("/opt/trn_rl_repo", "/root/.axon_site/_ro/trn_rl_repo"):
    if os.path.isdir(_p) and _p not in sys.path:
        sys.path.insert(0, _p)

import numpy as np
import ml_dtypes

import concourse.bass as bass
import concourse.bacc as bacc
import concourse.mybir as mybir
from concourse import tile
from concourse.vector_clock import ScopedClock
import bass_rust

BF16 = mybir.dt.bfloat16
FP32 = mybir.dt.float32
AF = mybir.ActivationFunctionType
ALU = mybir.AluOpType

N_CORES = 8
ROUTING_ITERS = 3


def _patch_tile_drain():
    """This walrus build allows only one sync wait per instruction; Tile's
    kernel-tail drain packs the whole vector clock onto one Drain.  Split it
    into one drain per outstanding proc."""

    def _drain_and_barrier(self, tick_clock, wait_clock):
        gc = tick_clock.global_clock
        ticks = eval(re.sub(r"VectorClock", "", repr(gc)))
        n = len(ticks)
        for i, v in enumerate(ticks):
            if v > 0:
                single = [0] * n
                single[i] = v
                d = self.nc.sync.drain()
                wait_clock.add_sem_waits(
                    d.ins, ScopedClock({None: bass_rust.VectorClock(single)})
                )
        self.nc.all_engine_barrier()
        popped = self.nc._tile_sem_poison_stack.pop()
        assert popped is self._sem_poison
        self.nc.clear_and_free_semaphores(list(self.sems.allocated().values()))
        self.nc.all_engine_barrier()

    tile.TileContext._drain_and_barrier = _drain_and_barrier


_patch_tile_drain()


def _dims(B, NLOC, I, J, O):
    JQ = 4
    JG = J // JQ
    PK = JQ * O            # stage-1 contraction rows
    NN = min(128 // I, NLOC)
    PC = NN * I            # (n,i) chunk partition rows
    NCH = NLOC // NN
    PN = min(128, NLOC)    # n rows per half
    NH = NLOC // PN
    CHH = NCH // NH
    assert JG * JQ == J and NN * NCH == NLOC and PN * NH == NLOC
    return JQ, JG, PK, NN, PC, NCH, PN, NH, CHH


def build_kernel(B=64, NLOC=256, I=16, J=32, O=32, n_cores=N_CORES):
    """Emit the per-core SPMD program.  Returns the Bass module."""
    JQ, JG, PK, NN, PC, NCH, PN, NH, CHH = _dims(B, NLOC, I, J, O)
    JO = J * O
    IJO = I * J * O

    nc = bacc.Bacc("TRN2", target_bir_lowering=False, debug=False,
                   num_devices=n_cores)

    # ---- dram parameters (host-prepped layouts) ----
    wb_d = nc.declare_dram_parameter("w_b", [JG * PK, NLOC * I], BF16,
                                     isOutput=False)
    wc_d = nc.declare_dram_parameter("w_c", [NLOC, IJO], BF16, isOutput=False)
    x2_d = nc.declare_dram_parameter("x2", [NLOC, I * B], BF16, isOutput=False)
    xin_d = nc.declare_dram_parameter("x_i_n", [NCH * PC, B], BF16,
                                      isOutput=False)
    ones_d = nc.declare_dram_parameter("ones_bd", [PC, NN], BF16,
                                       isOutput=False)
    out_d = nc.declare_dram_parameter("v_out", [B, JO], FP32, isOutput=True)

    # collective bounce buffers (internal dram)
    ar_space = "Shared" if n_cores > 4 else "Local"
    # all bounces are [B, JO]; t>=1 in bf16 (partials transposed on-chip
    # with the xbar before the AllReduce)
    ar_in = [nc.dram_tensor(f"ar_in{t}", [B, JO],
                            FP32 if t == 0 else BF16)
             for t in range(ROUTING_ITERS)]
    ar_out = [nc.dram_tensor(f"ar_out{t}", [B, JO],
                             FP32 if t == 0 else BF16, addr_space=ar_space)
              for t in range(ROUTING_ITERS)]
    blog_d = nc.dram_tensor("blog_spill", [NLOC, J * B], FP32)

    rg = [list(range(n_cores))]

    with tile.TileContext(nc) as tc:
        with (
            tc.tile_pool(name="wpool", bufs=1) as wpool,
            tc.tile_pool(name="small", bufs=1) as small,
            tc.tile_pool(name="work", bufs=2) as work,
            tc.tile_pool(name="cxp", bufs=2) as cxp,
            tc.tile_pool(name="xwvp", bufs=2) as xwvp,
            tc.tile_pool(name="achk", bufs=3) as achk,
            tc.tile_pool(name="ps", bufs=2, space="PSUM") as ps,
            tc.tile_pool(name="ps_wv", bufs=2, space="PSUM") as ps_wv,
            tc.tile_pool(name="ps_a", bufs=1, space="PSUM") as ps_a,
        ):
            # ---- resident tensors ----
            wc_t = []
            for h in range(NH):
                t = wpool.tile([PN, IJO], BF16, tag=f"wc{h}")
                nc.sync.dma_start(out=t[:, :], in_=wc_d[h * PN:(h + 1) * PN, :])
                wc_t.append(t)
            x2_t = []
            for h in range(NH):
                t = wpool.tile([PN, I * B], BF16, tag=f"x2_{h}")
                nc.sync.dma_start(out=t[:, :], in_=x2_d[h * PN:(h + 1) * PN, :])
                x2_t.append(t)
            xin_t = []
            for ch in range(NCH):
                t = wpool.tile([PC, B], BF16, tag=f"xin{ch}")
                nc.sync.dma_start(out=t[:, :], in_=xin_d[ch * PC:(ch + 1) * PC, :])
                xin_t.append(t)
            ones_t = wpool.tile([PC, NN], BF16, tag="ones")
            nc.sync.dma_start(out=ones_t[:, :], in_=ones_d[:, :])
            wb_t = []
            for jg in range(JG):
                t = wpool.tile([PK, NLOC * I], BF16, tag=f"wb{jg}")
                nc.sync.dma_start(out=t[:, :], in_=wb_d[jg * PK:(jg + 1) * PK, :])
                wb_t.append(t)

            # v-blockdiag rhs tiles for stage-1 (zero background, diag
            # rewritten per iteration)
            vblk = []
            for jg in range(JG):
                t = wpool.tile([PK, JQ * B], BF16, tag=f"vblk{jg}")
                nc.vector.memset(t[:, :], 0.0)
                vblk.append(t)

            v_bf = small.tile([B, JO], BF16, tag="v_bf")
            s_sb = small.tile([B, JO], FP32, tag="s_sb")

            # ---------- squash: s_sb -> v (bf16) ----------
            def squash(final=False):
                sq = work.tile([B, JO], FP32, tag="sq", bufs=1)
                nc.vector.tensor_tensor(out=sq[:, :], in0=s_sb[:, :],
                                        in1=s_sb[:, :], op=ALU.mult)
                norm = work.tile([B, J], FP32, tag="norm")
                nc.vector.tensor_reduce(
                    out=norm[:, :].unsqueeze(2),
                    in_=sq[:, :].rearrange("p (j o) -> p j o", o=O),
                    axis=mybir.AxisListType.X, op=ALU.add)
                np1 = work.tile([B, J], FP32, tag="np1")
                nc.vector.tensor_scalar_add(out=np1[:, :], in0=norm[:, :],
                                            scalar1=1.0)
                r1 = work.tile([B, J], FP32, tag="r1")
                nc.vector.reciprocal(out=r1[:, :], in_=np1[:, :])
                ne = work.tile([B, J], FP32, tag="ne")
                nc.vector.tensor_scalar_add(out=ne[:, :], in0=norm[:, :],
                                            scalar1=1e-8)
                sr = work.tile([B, J], FP32, tag="sr")
                nc.scalar.activation(sr[:, :], ne[:, :], AF.Sqrt)
                r2 = work.tile([B, J], FP32, tag="r2")
                nc.vector.reciprocal(out=r2[:, :], in_=sr[:, :])
                sc = work.tile([B, J], FP32, tag="sc")
                nc.vector.tensor_tensor(out=sc[:, :], in0=norm[:, :],
                                        in1=r1[:, :], op=ALU.mult)
                nc.vector.tensor_tensor(out=sc[:, :], in0=sc[:, :],
                                        in1=r2[:, :], op=ALU.mult)
                sc_b = sc[:, :].unsqueeze(2).broadcast_to((B, J, O))
                s3 = s_sb[:, :].rearrange("p (j o) -> p j o", o=O)
                if final:
                    vf = work.tile([B, JO], FP32, tag="sq", bufs=1, name="vf")
                    nc.vector.tensor_tensor(
                        out=vf[:, :].rearrange("p (j o) -> p j o", o=O),
                        in0=s3, in1=sc_b, op=ALU.mult)
                    nc.sync.dma_start(out=out_d[:, :], in_=vf[:, :])
                else:
                    nc.vector.tensor_tensor(
                        out=v_bf[:, :].rearrange("p (j o) -> p j o", o=O),
                        in0=s3, in1=sc_b, op=ALU.mult)

            # ---------- iter 0: s0 ----------
            nsplit = 512 if JO > 512 else JO
            s0_ps = [ps.tile([B, nsplit], FP32, tag="sps", bufs=2, name=f"s0ps{k}")
                     for k in range(JO // nsplit)]
            first = True
            for h in range(NH):
                for i in range(I):
                    lhsT = x2_t[h][:, i * B:(i + 1) * B]
                    for k in range(JO // nsplit):
                        nc.tensor.matmul(
                            s0_ps[k][:, :], lhsT,
                            wc_t[h][:, i * JO + k * nsplit: i * JO + (k + 1) * nsplit],
                            start=first, stop=(h == NH - 1 and i == I - 1))
                    first = False
            for k in range(JO // nsplit):
                nc.vector.tensor_scalar_mul(
                    out=s_sb[:, k * nsplit:(k + 1) * nsplit],
                    in0=s0_ps[k][:, :], scalar1=1.0 / J)
            nc.sync.dma_start(out=ar_in[0][:, :], in_=s_sb[:, :])
            nc.gpsimd.collective_compute(
                "AllReduce", ALU.add, replica_groups=rg,
                ins=[ar_in[0][:, :]], outs=[ar_out[0][:, :]])
            nc.sync.dma_start(out=s_sb[:, :], in_=ar_out[0][:, :])
            squash()

            # ---------- routing iterations ----------
            for t in range(1, ROUTING_ITERS):
                # v -> transposed slices -> block-diag rhs tiles
                for jg in range(JG):
                    vT = work.tile([PK, B], BF16, tag="vT", bufs=2,
                                   name=f"vT{t}_{jg}")
                    nc.sync.dma_start_transpose(
                        vT[:, :], v_bf[:, jg * PK:(jg + 1) * PK])
                    for jq in range(JQ):
                        nc.vector.tensor_copy(
                            out=vblk[jg][jq * O:(jq + 1) * O,
                                         jq * B:(jq + 1) * B],
                            in_=vT[jq * O:(jq + 1) * O, :])

                # agreement logits per n-half.  Chunks processed in pairs:
                # stage-2 matmuls for the two chunks run concurrently in
                # separate PE column-groups (tile_position), and one wide
                # ScalarE copy evacuates both.
                QW = min(4, JG)          # j-groups per wide psum tile
                evi = 0
                c_bf = []
                for h in range(NH):
                    asb = work.tile([PN, J * B], FP32, tag="asb", bufs=2,
                                    name=f"asb{t}_{h}")
                    for chh0 in range(0, CHH, 2):
                        npack = min(2, CHH - chh0)
                        xwvs = []
                        for pi in range(npack):
                            chh = chh0 + pi
                            ch = h * CHH + chh
                            for qw in range(JG // QW):
                                wv_ps = ps_wv.tile(
                                    [PC, QW * JQ * B], FP32, tag="wvps",
                                    bufs=2, name=f"wvps{t}_{ch}_{qw}")
                                for jj in range(QW):
                                    jg = qw * QW + jj
                                    nc.tensor.matmul(
                                        wv_ps[:, jj * JQ * B:(jj + 1) * JQ * B],
                                        wb_t[jg][:, ch * PC:(ch + 1) * PC],
                                        vblk[jg][:, :], start=True, stop=True)
                                # evacuate + multiply by x.  Rotate the
                                # evac across engines: most go straight
                                # through DVE (TT from PSUM); some bounce
                                # via a ScalarE cast-copy so GpSimd/DVE can
                                # do the multiply from SBUF at 2x.
                                xwv = xwvp.tile([PC, QW * JQ * B], BF16,
                                                tag="xwv", bufs=3,
                                                name=f"xwv{t}_{ch}_{qw}")
                                xb = xin_t[ch][:, :].unsqueeze(1).unsqueeze(1) \
                                    .broadcast_to((PC, QW, JQ, B))
                                xwv4 = xwv[:, :].rearrange(
                                    "p (g q b) -> p g q b", q=JQ, b=B)
                                mode = evi % 8
                                evi += 1
                                if mode >= 7:
                                    wvs = xwvp.tile([PC, QW * JQ * B], BF16,
                                                    tag="wvs", bufs=2,
                                                    name=f"wvs{t}_{ch}_{qw}")
                                    nc.scalar.activation(wvs[:, :], wv_ps[:, :],
                                                         AF.Copy)
                                    meng = nc.vector
                                    meng.tensor_tensor(
                                        out=xwv4,
                                        in0=wvs[:, :].rearrange(
                                            "p (g q b) -> p g q b", q=JQ, b=B),
                                        in1=xb, op=ALU.mult)
                                else:
                                    wv4 = wv_ps[:, :].rearrange(
                                        "p (g q b) -> p g q b", q=JQ, b=B)
                                    nc.vector.tensor_tensor(
                                        out=xwv4, in0=wv4, in1=xb, op=ALU.mult)
                                xwvs.append(xwv)
                        # stage-2: ones-blockdiag reduce over i; the pack's
                        # chunks land in different psum partition groups
                        nw = JG * JQ * B
                        ksplit = min(1024, nw)
                        ap_rows = 64 if npack == 2 else NN
                        for k in range(nw // ksplit):
                            a_ps = ps_a.tile([ap_rows, ksplit], FP32,
                                             tag="aps", bufs=1,
                                             name=f"aps{t}_{h}_{chh0}_{k}")
                            msplit = min(512, ksplit)
                            for pi in range(npack):
                                for m in range(ksplit // msplit):
                                    cof = k * ksplit + m * msplit
                                    xw = xwvs[pi * (JG // QW) + cof // (QW * JQ * B)]
                                    koff = cof % (QW * JQ * B)
                                    nc.tensor.matmul(
                                        a_ps[32 * pi:32 * pi + NN,
                                             m * msplit:(m + 1) * msplit],
                                        ones_t[:, :],
                                        xw[:, koff:koff + msplit],
                                        start=True, stop=True,
                                        tile_position=(0, 32 * pi) if pi else None)
                            ach = achk.tile([ap_rows, ksplit], FP32,
                                            tag="ach", bufs=2,
                                            name=f"ach{t}_{h}_{chh0}_{k}")
                            nc.scalar.activation(ach[:, :], a_ps[:, :],
                                                 AF.Copy)
                            for pi in range(npack):
                                chh = chh0 + pi
                                nc.sync.dma_start(
                                    out=asb[chh * NN:(chh + 1) * NN,
                                            k * ksplit:(k + 1) * ksplit],
                                    in_=ach[32 * pi:32 * pi + NN, :])
                    if t == 1:
                        nc.sync.dma_start(
                            out=blog_d[h * PN:(h + 1) * PN, :], in_=asb[:, :])
                    else:
                        bl = work.tile([PN, J * B], FP32, tag="asb", bufs=2,
                                       name=f"bl{t}_{h}")
                        nc.sync.dma_start(
                            out=bl[:, :], in_=blog_d[h * PN:(h + 1) * PN, :])
                        nc.vector.tensor_tensor(out=asb[:, :], in0=asb[:, :],
                                                in1=bl[:, :], op=ALU.add)
                    # softmax over j (no max subtraction; logits are O(1));
                    # exp in place, bf16 scratch for the denominator tree
                    nc.scalar.activation(asb[:, :], asb[:, :], AF.Exp)
                    cb = work.tile([PN, J * B], BF16, tag="cb", bufs=2,
                                   name=f"cb{t}_{h}")
                    nc.vector.tensor_copy(out=cb[:, :], in_=asb[:, :])
                    w = J
                    while w > 1:
                        hw = w // 2
                        s3 = cb[:, :].rearrange("p (j b) -> p j b", b=B)
                        nc.vector.tensor_tensor(
                            out=s3[:, 0:hw, :], in0=s3[:, 0:hw, :],
                            in1=s3[:, hw:w, :], op=ALU.add)
                        w = hw
                    re_ = work.tile([PN, B], FP32, tag="re", bufs=2,
                                    name=f"re{t}_{h}")
                    nc.vector.reciprocal(
                        out=re_[:, :],
                        in_=cb[:, :].rearrange("p (j b) -> p j b", b=B)[:, 0:1, :].squeeze(1))
                    nc.vector.tensor_tensor(
                        out=cb[:, :].rearrange("p (j b) -> p j b", b=B),
                        in0=asb[:, :].rearrange("p (j b) -> p j b", b=B),
                        in1=re_[:, :].unsqueeze(1).broadcast_to((PN, J, B)),
                        op=ALU.mult)
                    c_bf.append(cb)

                # s_t = sum_{n,i} cx * W   (cx built in i-halves, build work
                # split between VectorE and GpSimd)
                IH = I // 2 if I % 2 == 0 else I
                cxi = 0
                for jg in range(JG):
                    s_ps = ps.tile([PK, JQ * B], FP32, tag="sps", bufs=2,
                                   name=f"sps{t}_{jg}")
                    for h in range(NH):
                        for ih in range(I // IH):
                            cx = cxp.tile([PN, IH * JQ * B], BF16, tag="cx",
                                          bufs=2, name=f"cx{t}_{jg}_{h}_{ih}")
                            i0 = ih * IH
                            c_ap = c_bf[h][:, :].rearrange(
                                "p (j b) -> p j b", b=B)[:, jg * JQ:(jg + 1) * JQ, :]
                            c_ap = c_ap.unsqueeze(1).broadcast_to(
                                (PN, IH, JQ, B))
                            x_ap = x2_t[h][:, i0 * B:(i0 + IH) * B].rearrange(
                                "p (i b) -> p i b", b=B).unsqueeze(2).broadcast_to(
                                (PN, IH, JQ, B))
                            eng = nc.gpsimd if cxi % 3 == 2 else nc.vector
                            cxi += 1
                            eng.tensor_tensor(
                                out=cx[:, :].rearrange(
                                    "p (i q b) -> p i q b", q=JQ, b=B),
                                in0=c_ap, in1=x_ap, op=ALU.mult)
                            for ii in range(IH):
                                i = i0 + ii
                                nc.tensor.matmul(
                                    s_ps[:, :],
                                    wc_t[h][:, i * JO + jg * PK: i * JO + jg * PK + PK],
                                    cx[:, ii * JQ * B:(ii + 1) * JQ * B],
                                    start=(h == 0 and i == 0),
                                    stop=(h == NH - 1 and i == I - 1))
                    # extract diagonal blocks into a jg-pair tile, then
                    # xbar-transpose to [B, (j,o)] for the AllReduce bounce
                    if jg % 2 == 0:
                        sdp = work.tile([PK, 2 * B], BF16, tag="sd", bufs=2,
                                        name=f"sd{t}_{jg}")
                    half = (jg % 2) * B
                    for jq in range(JQ):
                        nc.vector.tensor_copy(
                            out=sdp[jq * O:(jq + 1) * O, half:half + B],
                            in_=s_ps[jq * O:(jq + 1) * O,
                                     jq * B:(jq + 1) * B])
                    if jg % 2 == 1 or jg == JG - 1:
                        sdT = work.tile([2 * B, PK], BF16, tag="sdT", bufs=2,
                                        name=f"sdT{t}_{jg}")
                        nc.sync.dma_start_transpose(sdT[:, :], sdp[:, :])
                        for g2 in range(jg % 2 + 1):
                            jgw = jg - (jg % 2) + g2
                            nc.sync.dma_start(
                                out=ar_in[t][:, jgw * PK:(jgw + 1) * PK],
                                in_=sdT[g2 * B:(g2 + 1) * B, :])
                nc.gpsimd.collective_compute(
                    "AllReduce", ALU.add, replica_groups=rg,
                    ins=[ar_in[t][:, :]], outs=[ar_out[t][:, :]])
                sT = work.tile([B, JO], BF16, tag="sT", bufs=1,
                               name=f"sT{t}")
                nc.sync.dma_start(out=sT[:, :], in_=ar_out[t][:, :])
                nc.vector.tensor_copy(out=s_sb[:, :], in_=sT[:, :])
                squash(final=(t == ROUTING_ITERS - 1))

    nc.compile()
    return nc


def prep_inputs(x, weights, n_cores=N_CORES):
    """Shard + lay out the inputs for each core."""
    B, N, I = x.shape
    _, J, O, _ = weights.shape
    NLOC = N // n_cores
    JQ, JG, PK, NN, PC, NCH, PN, NH, CHH = _dims(B, NLOC, I, J, O)
    bf = ml_dtypes.bfloat16
    ones = np.zeros((PC, NN), dtype=bf)
    for nn in range(NN):
        ones[nn * I:(nn + 1) * I, nn] = 1.0
    x = np.asarray(x, dtype=np.float32)
    weights = np.asarray(weights, dtype=np.float32)
    in_maps = []
    for r in range(n_cores):
        n0 = r * NLOC
        Wr = weights[n0:n0 + NLOC]              # [NLOC, J, O, I]
        xr = x[:, n0:n0 + NLOC, :]              # [B, NLOC, I]
        w_b = Wr.reshape(NLOC, JG, JQ, O, I).transpose(1, 2, 3, 0, 4) \
            .reshape(JG * JQ * O, NLOC * I).astype(bf)
        w_c = Wr.transpose(0, 3, 1, 2).reshape(NLOC, I * J * O).astype(bf)
        x_nib = xr.transpose(1, 2, 0)           # [NLOC, I, B]
        x2 = x_nib.reshape(NLOC, I * B).astype(bf)
        x_i_n = x_nib.reshape(NCH * PC, B).astype(bf)
        in_maps.append({
            "w_b": np.ascontiguousarray(w_b),
            "w_c": np.ascontiguousarray(w_c),
            "x2": np.ascontiguousarray(x2),
            "x_i_n": np.ascontiguousarray(x_i_n),
            "ones_bd": ones,
        })
    return in_maps


_CACHE = {}


def kernel(x, weights):
    from concourse.bass_utils import run_bass_kernel_spmd
    x = np.asarray(x)
    weights = np.asarray(weights)
    B, N, I = x.shape
    _, J, O, _ = weights.shape
    NLOC = N // N_CORES
    key = (B, N, I, J, O)
    if key not in _CACHE:
        _CACHE[key] = build_kernel(B=B, NLOC=NLOC, I=I, J=J, O=O)
    nc = _CACHE[key]
    in_maps = prep_inputs(x, weights)
    res = run_bass_kernel_spmd(nc, in_maps, list(range(N_CORES)))
    out = np.asarray(res.results[0]["v_out"], dtype=np.float32)
    return out.reshape(B, J, O)
